# revision 16
# baseline (speedup 1.0000x reference)
"""DenseMPNN Trainium2 kernel (8-core SPMD, batch data-parallel), v3.

Strategy:
- Shard batch B=32 across 8 cores (4 molecules/core); replicate weights.
- Host packs each molecule's ~4%-dense adjacency into an UNDIRECTED edge
  list (E_u <= 128): partition row e holds both directions of undirected
  edge {v,w} (fwd = v->w, bwd = w->v).
    H0[e,d] = relu(X[:,d,e]^T @ Wi)          X = [atoms[src]; bonds]
    iter:  HWh_d = H_d @ Wh                  (PE transpose + matmul)
           Q_d  = M_d0@HWh_0 + M_d1@HWh_1 + I@H0_d
           H_d  = relu(Q_d)
    out = relu(atoms@Wo_a + agg_final@Wo_h + bo)
  where M_de = inv_d (.) (G1_d @ T_e^T) - [e==1-d] diag(inv_d) are
  host-built [E,E] edge->edge message matrices: they fold the node
  aggregation (T), the source gather (G1), the reverse-edge subtraction
  and the 1/n_nbr scaling into ONE stationary operand, so the whole
  per-iteration update is matmuls + a single relu. This removes the
  [N,H] P round trip and the DVE-only scalar_tensor_tensor that
  bottlenecked v2 (Pool cannot access PSUM; Act has no stt).
- bf16 data (f32 PSUM accumulate); PE clock warmed up by dummy matmuls
  during the DMA phase; Wi loaded via Pool/SWDGE in parallel with the
  SP/HWDGE page loads; PSUM->SBUF copies split across DVE and Act.
"""

import numpy as np

_B, _N, _A, _EB, _H = 32, 64, 133, 14, 256
_DEPTH = 3
_NCORES = 8
_MPC = _B // _NCORES  # molecules per core
_KX = _A + _EB  # 147

_cache = {}
_NWARM = 64  # PE clock-ramp warmup matmuls
_FP8_HWH = True  # H@Wh in fp8e4m3 DoubleRow (2x PE rate, one matmul per dir)


def _build_nc(E_u, reps=1):
    import sys
    for p in ("/opt/trn_rl_repo",):
        if p not in sys.path:
            sys.path.insert(0, p)
    import concourse.bass as bass  # noqa: F401
    import concourse.mybir as mybir
    import concourse.tile as tile
    from concourse import bacc
    from concourse.masks import make_identity

    BF = mybir.dt.bfloat16
    F8 = mybir.dt.float8e4
    F32 = mybir.dt.float32
    HT_N = _H // 128  # hidden chunks of 128
    RELU = mybir.ActivationFunctionType.Relu
    DR = mybir.MatmulPerfMode.DoubleRow

    E2 = 2 * E_u
    XC = 2 * E2 + 2                 # X1 | X2 | inv  columns per molecule
    GC = 64 + 64 + 2 * _N + 4 * E_u  # aT1 | aT2 | tm | M  columns per molecule

    nc = bacc.Bacc(None, target_bir_lowering=False, debug=False)

    # --- DRAM I/O (bf16 pages, ordered by first use) ---
    mx_d = nc.dram_tensor("mx", [_MPC, 128, XC], BF, kind="ExternalInput")
    wi_d = nc.dram_tensor("wi", [128, 512], BF, kind="ExternalInput")
    if _FP8_HWH:
        wh_d = nc.dram_tensor("wh", [128, 512], F8, kind="ExternalInput")
    else:
        wh_d = nc.dram_tensor("wh", [128, 512], BF, kind="ExternalInput")
    mg_d = nc.dram_tensor("mg", [_MPC, 128, GC], BF, kind="ExternalInput")
    wo_d = nc.dram_tensor("wo", [128, 1024], BF, kind="ExternalInput")
    out_d = nc.dram_tensor("out", [_MPC, _N, _H], F32, kind="ExternalOutput")

    with tile.TileContext(nc) as tc:
        import contextlib
        with contextlib.ExitStack() as ctx:
            consts = ctx.enter_context(tc.tile_pool(name="consts", bufs=1))
            work = ctx.enter_context(tc.tile_pool(name="work", bufs=4))
            hbuf = ctx.enter_context(tc.tile_pool(name="hbuf", bufs=4))
            ps_mm = ctx.enter_context(tc.tile_pool(name="ps_mm", bufs=5, space="PSUM"))
            ps_tr = ctx.enter_context(tc.tile_pool(name="ps_tr", bufs=2, space="PSUM"))

            # ---- PE warmup (independent of all loads): ramp the PE clock
            # from 0.65 GHz to 2.4 GHz while the DMAs fly. ----
            warm = consts.tile([128, 64], BF)
            nc.vector.memset(warm, 0.0)
            ps_w = ps_tr.tile([64, 64], F32, tag="tr", name="ps_w")
            for i in range(_NWARM):
                nc.tensor.matmul(ps_w, warm, warm[:, 0:64], start=True, stop=True)

            # ---- loads: SP/HWDGE pages in first-use order ----
            wi_s = consts.tile([128, 512], BF)
            nc.sync.dma_start(out=wi_s, in_=wi_d[:, :])
            mxa_s = consts.tile([128, 2, XC], BF)
            nc.sync.dma_start(out=mxa_s, in_=mx_d[0:2].rearrange("m p c -> p m c"))
            mxb_s = consts.tile([128, 2, XC], BF)
            nc.sync.dma_start(out=mxb_s, in_=mx_d[2:4].rearrange("m p c -> p m c"))
            wh_s = consts.tile([128, HT_N, 256], F8 if _FP8_HWH else BF)
            nc.sync.dma_start(out=wh_s, in_=wh_d.rearrange("p (c n) -> p c n", c=HT_N))
            mg_s = consts.tile([128, _MPC, GC], BF)
            nc.sync.dma_start(out=mg_s, in_=mg_d.rearrange("m p c -> p m c"))
            wo_s = consts.tile([128, 1024], BF)
            nc.sync.dma_start(out=wo_s, in_=wo_d[:, :])

            # ---- small consts (Pool, after its SWDGE issue) ----
            ident = consts.tile([128, 128], BF)
            make_identity(nc, ident)

            wi1 = wi_s[:, 0:256]
            wi2 = wi_s[0:_KX - 128, 256:512]
            woa1 = wo_s[:, 0:256]
            woa2 = wo_s[0:_A + 1 - 128, 256:512]

            def mslice(m):
                mx = mxa_s if m < 2 else mxb_s
                sl = m % 2
                s = {}
                s["X1"] = mx[:, sl, 0:E2].rearrange("p (d e) -> p d e", d=2)
                s["X2"] = mx[0:_KX - 128, sl, E2:2 * E2].rearrange(
                    "p (d e) -> p d e", d=2)
                s["aT1"] = mg_s[:, m, 0:64]
                s["aT2"] = mg_s[0:_A + 1 - 128, m, 64:128]
                s["tm"] = mg_s[0:E_u, m, 128:128 + 2 * _N].rearrange(
                    "p (d n) -> p d n", d=2)
                s["M"] = mg_s[0:E_u, m, 128 + 2 * _N:GC].rearrange(
                    "p (j e) -> p j e", j=4)  # j = 2*d + e
                return s

            def vrelu(eng, out, in_):
                if eng == 0:
                    nc.vector.tensor_scalar_max(out=out, in0=in_, scalar1=0.0)
                else:
                    nc.scalar.activation(out=out, in_=in_, func=RELU)

            for rep in range(reps):
                S = [mslice(m) for m in range(_MPC)]

                # ---- H0 = relu(X^T @ Wi)  [E_u, 2, H] ----
                for m in range(_MPC):
                    ps_h0 = ps_mm.tile([E_u, 2, _H], F32, tag="mm", name=f"psh0{m}")
                    for d in range(2):
                        nc.tensor.matmul(ps_h0[:, d, :], S[m]["X1"][:, d, :], wi1,
                                         start=True, stop=False)
                        nc.tensor.matmul(ps_h0[:, d, :], S[m]["X2"][:, d, :], wi2,
                                         start=False, stop=True)
                    S[m]["ps_h0"] = ps_h0
                for m in range(_MPC):
                    h0 = hbuf.tile([E_u, 2, _H], BF, tag="h0", name=f"h0_{m}")
                    vrelu(m % 2, h0, S[m]["ps_h0"])
                    S[m]["h0"] = h0
                    S[m]["h"] = h0  # initial H == H0 (mask folded into packing)

                # ---- message passing iterations ----
                for it in range(_DEPTH - 1):
                    for m in range(_MPC):
                        ps_t = ps_tr.tile([128, HT_N, 2, E_u], BF, tag="tr",
                                          name=f"pst{m}")
                        h = S[m]["h"]
                        for hh in range(HT_N):
                            for d in range(2):
                                nc.tensor.transpose(
                                    ps_t[:, hh, d, :],
                                    h[:, d, hh * 128:(hh + 1) * 128],
                                    ident[:E_u, :E_u])
                        S[m]["ps_t"] = ps_t
                    for m in range(_MPC):
                        ht = work.tile([128, HT_N, 2, E_u],
                                       F8 if _FP8_HWH else BF,
                                       tag="ht", name=f"ht{m}")
                        nc.vector.tensor_copy(out=ht, in_=S[m]["ps_t"])
                        S[m]["ht"] = ht
                    for m in range(_MPC):
                        ps_hw = ps_mm.tile([E_u, 2, _H], F32, tag="mm",
                                           name=f"pshw{m}")
                        for d in range(2):
                            if _FP8_HWH:
                                # DoubleRow: both 128-row K-chunks in one
                                # matmul (slot dim = hh on both operands)
                                nc.tensor.matmul(ps_hw[:, d, :],
                                                 S[m]["ht"][:, :, d, :],
                                                 wh_s[:, :, :],
                                                 start=True, stop=True,
                                                 perf_mode=DR)
                            else:
                                for hh in range(HT_N):
                                    nc.tensor.matmul(ps_hw[:, d, :],
                                                     S[m]["ht"][:, hh, d, :],
                                                     wh_s[:, hh, :],
                                                     start=(hh == 0),
                                                     stop=(hh == HT_N - 1))
                        S[m]["ps_hw"] = ps_hw
                    for m in range(_MPC):
                        hwh = work.tile([E_u, 2, _H], BF, tag="hwh", name=f"hwh{m}")
                        nc.scalar.copy(out=hwh, in_=S[m]["ps_hw"])
                        S[m]["hwh"] = hwh
                    # Q_d = M_d0 @ HWh_0 + M_d1 @ HWh_1 + H0_d  (one PSUM group)
                    for m in range(_MPC):
                        ps_q = ps_mm.tile([E_u, 2, _H], F32, tag="mm",
                                          name=f"psq{m}")
                        nc.tensor.matmul(ps_q, ident[:E_u, :E_u],
                                         S[m]["h0"].rearrange("e d h -> e (d h)"),
                                         start=True, stop=False,
                                         skip_group_check=True)
                        for d in range(2):
                            for e in range(2):
                                nc.tensor.matmul(ps_q[:, d, :],
                                                 S[m]["M"][:, 2 * d + e, :],
                                                 S[m]["hwh"][:, e, :],
                                                 start=False,
                                                 stop=(d == 1 and e == 1),
                                                 skip_group_check=True)
                        S[m]["ps_q"] = ps_q
                    HN_ENG = [0, 1, 1, 0]
                    for m in range(_MPC):
                        hn = hbuf.tile([E_u, 2, _H], BF, tag="hn", name=f"hn{m}")
                        vrelu(HN_ENG[m], hn, S[m]["ps_q"])
                        S[m]["h"] = hn

                # ---- readout ----
                for m in range(_MPC):
                    ps_a = ps_tr.tile([128, HT_N, _N], F32, tag="tr", name=f"psa{m}")
                    h = S[m]["h"]
                    for hh in range(HT_N):
                        for d in range(2):
                            nc.tensor.matmul(ps_a[:, hh, :],
                                             h[:, d, hh * 128:(hh + 1) * 128],
                                             S[m]["tm"][:, d, :],
                                             start=(d == 0), stop=(d == 1))
                    S[m]["ps_a"] = ps_a
                for m in range(_MPC):
                    af = work.tile([128, HT_N, _N], BF, tag="af", name=f"af{m}")
                    nc.vector.tensor_copy(out=af, in_=S[m]["ps_a"])
                    S[m]["af"] = af
                for m in range(_MPC):
                    ps_o = ps_mm.tile([_N, _H], F32, tag="mm", name=f"pso{m}")
                    nc.tensor.matmul(ps_o, S[m]["aT1"], woa1, start=True, stop=False)
                    nc.tensor.matmul(ps_o, S[m]["aT2"], woa2, start=False, stop=False)
                    for hh in range(HT_N):
                        nc.tensor.matmul(ps_o, S[m]["af"][:, hh, :],
                                         wo_s[:, 512 + hh * 256:512 + (hh + 1) * 256],
                                         start=False, stop=(hh == HT_N - 1))
                    S[m]["ps_o"] = ps_o
                for m in range(_MPC):
                    o_s = work.tile([_N, _H], F32, tag="o", name=f"o{m}")
                    vrelu(m % 2, o_s, S[m]["ps_o"])
                    nc.sync.dma_start(out=out_d[m], in_=o_s)

    nc.compile()
    return nc


def _prep_inputs(atoms, bonds, adj, Wi, Wh, Wo, bo):
    import ml_dtypes
    BF = np.dtype(ml_dtypes.bfloat16)
    B, N, A = atoms.shape
    H = Wh.shape[0]

    und = []
    for b in range(B):
        vw = np.argwhere(np.triu(adj[b]) > 0)  # canonical (v < w)
        und.append(vw)
    E_max = max(len(e) for e in und)
    E_u = max(32, ((E_max + 31) // 32) * 32)
    assert E_u <= 128, f"E_u={E_u} exceeds one partition tile"

    E2 = 2 * E_u
    XC = 2 * E2 + 2
    GC = 64 + 64 + 2 * N + 4 * E_u
    mx = np.zeros((B, 128, XC), np.float32)
    mg = np.zeros((B, 128, GC), np.float32)

    for b in range(B):
        vw = und[b]
        E = len(vw)
        v_e, w_e = vw[:, 0], vw[:, 1]
        deg = adj[b].sum(1)
        ar = np.arange(E)

        # X[:, d, e] = [atoms[src(e,d)] ; bonds(e,d)]  (KX = 133+14 rows)
        X = np.zeros((_KX, 2, E_u), np.float32)
        X[:A, 0, :E] = atoms[b, v_e].T
        X[:A, 1, :E] = atoms[b, w_e].T
        X[A:, 0, :E] = bonds[b, v_e, w_e].T
        X[A:, 1, :E] = bonds[b, w_e, v_e].T
        mx[b, :, 0:E2] = X[0:128].reshape(128, E2)
        mx[b, 0:_KX - 128, E2:2 * E2] = X[128:].reshape(_KX - 128, E2)
        inv = np.zeros((E_u, 2), np.float32)
        inv[:E, 0] = 1.0 / np.maximum(deg[v_e] - 1.0, 1.0)
        inv[:E, 1] = 1.0 / np.maximum(deg[w_e] - 1.0, 1.0)
        mx[b, 0:E_u, 2 * E2:2 * E2 + 2] = inv  # kept for reference/debug

        atomsT = np.zeros((A + 1, N), np.float32)
        atomsT[:A] = atoms[b].T
        atomsT[A] = 1.0
        src = np.zeros((2, E_u), np.int64)  # src node of edge (d, e)
        tgt = np.zeros((2, E_u), np.int64)  # tgt node of edge (d, e)
        src[0, :E], src[1, :E] = v_e, w_e
        tgt[0, :E], tgt[1, :E] = w_e, v_e
        Tfb = np.zeros((E_u, 2, N), np.float32)
        Tfb[ar, 0, w_e] = 1.0
        Tfb[ar, 1, v_e] = 1.0
        # M_de[e1,e2] = inv_d[e1] * [src_d(e1) == tgt_e(e2)]
        #   - [e == 1-d] inv_d[e1] * [e1 == e2]
        # stored transposed (lhsT layout): band[:, 2d+e, :][e2, e1] = M_de[e1, e2]
        Mband = np.zeros((E_u, 4, E_u), np.float32)
        for d in range(2):
            for e in range(2):
                Mde = (src[d][:, None] == tgt[e][None, :]).astype(np.float32)
                if E < E_u:
                    Mde[E:, :] = 0.0
                    Mde[:, E:] = 0.0
                Mde *= inv[:, d][:, None]
                if e == 1 - d:
                    Mde -= np.diag(inv[:, d])
                Mband[:, 2 * d + e, :] = Mde.T
        mg[b, 0:128, 0:64] = atomsT[0:128]
        mg[b, 0:A + 1 - 128, 64:128] = atomsT[128:]
        mg[b, 0:E_u, 128:128 + 2 * N] = Tfb.reshape(E_u, 2 * N)
        mg[b, 0:E_u, 128 + 2 * N:GC] = Mband.reshape(E_u, 4 * E_u)

    wi = np.zeros((128, 512), np.float32)
    wi[:, 0:256] = Wi[0:128]
    wi[0:_KX - 128, 256:512] = Wi[128:]
    wh = Wh.reshape(2, 128, 256).transpose(1, 0, 2).reshape(128, 512)
    wo = np.zeros((128, 1024), np.float32)
    wo[:, 0:256] = Wo[0:128]
    wo[0:A + 1 - 128, 256:512] = np.concatenate([Wo[128:A], bo[None, :]], axis=0)
    wo[:, 512:1024] = Wo[A:].reshape(2, 128, 256).transpose(1, 0, 2).reshape(128, 512)

    F8 = np.dtype(ml_dtypes.float8_e4m3)
    shared = {
        "wi": wi.astype(BF),
        "wh": np.ascontiguousarray(wh).astype(F8 if _FP8_HWH else BF),
        "wo": wo.astype(BF),
    }

    def shard(x):
        return x.reshape((_NCORES, _MPC) + x.shape[1:])

    mx8, mg8 = shard(mx.astype(BF)), shard(mg.astype(BF))
    per_core = [
        {"mx": mx8[c], "mg": mg8[c], **shared}
        for c in range(_NCORES)
    ]
    return per_core, E_u


def kernel(atoms, bonds, adj, Wi, Wh, Wo, bo, _trace=False):
    import sys
    for p in ("/opt/trn_rl_repo",):
        if p not in sys.path:
            sys.path.insert(0, p)
    from concourse.bass_utils import run_bass_kernel_spmd

    atoms = np.asarray(atoms, np.float32)
    bonds = np.asarray(bonds, np.float32)
    adj = np.asarray(adj, np.float32)
    Wi = np.asarray(Wi, np.float32)
    Wh = np.asarray(Wh, np.float32)
    Wo = np.asarray(Wo, np.float32)
    bo = np.asarray(bo, np.float32)

    in_maps, E_u = _prep_inputs(atoms, bonds, adj, Wi, Wh, Wo, bo)

    key = ("nc", E_u)
    if key not in _cache:
        _cache[key] = _build_nc(E_u)
    nc = _cache[key]

    res = run_bass_kernel_spmd(nc, in_maps, list(range(_NCORES)), trace=_trace)
    outs = [res.results[c]["out"] for c in range(_NCORES)]
    full = np.concatenate(outs, axis=0).reshape(_B, _N, _H).astype(np.float32)
    if _trace:
        return full, res
    return full
